# revision 8
# baseline (speedup 1.0000x reference)
"""Trainium2 Bass kernel for nn_LocationEncoder (L1-distance attention).

Math (per batch element b):
    key   = ctx_c @ W_ctx + b_ctx                  [C, H]
    query = tgt_c @ W_ctx + b_ctx                  [T, H]
    value = [ctx_c, ctx_y] @ W_in + b_in           [C, H]
    S[t, c]  = sum_h |0.5*(key[c,h] - query[t,h])|  (= -score, always >= 0)
    w = softmax(-S, axis=c);  out = (w @ value) @ W_tgt + b_tgt

Sharding: 8 cores = (4 batches) x (2 halves of T).  Each core handles
T_LOC = 256 targets against the full C = 512 context of its batch.

Per-core device strategy (heavy axis h=128 on partitions):
  - keyT [H, C] and qT [H, T_LOC] via tiny matmuls (bias folded in through
    an appended ones-row on the moving operand; the b_ctx bias cancels in
    key - query anyway).
  - per target t:  absd = |0.5*keyT + (-0.5*qT[:, t])|  as bf16 [h, c],
    produced by ScalarE (activation Abs, per-partition bias) for some t's
    and VectorE (tensor_scalar fused add + abs_max, 4x mode) for the rest.
  - h-reduction on TensorE with absd as the *stationary* operand (FWL) and
    ones[128,1] moving: 4 chunk-matmuls -> S^T column [c, t] in PSUM.
    Scores are <= 0 so exp never overflows: NO max-subtraction needed.
  - per 128-target block: e^T = Exp(-S^T) (one ACT op), Z via ones-matmul
    (eT stationary), rep^T = value^T-style accumulation (value stationary,
    eT moving), out = rep^T.T @ W_tgt, then one fused VectorE
    scalar_tensor_tensor: out = psum * invZ[t] + b_tgt.
"""

import numpy as np
import ml_dtypes

import concourse.bass as bass
import concourse.mybir as mybir
import concourse.tile as tile
from concourse import bacc
from concourse.bass_utils import run_bass_kernel_spmd

B, C_CTX, T, H = 4, 512, 512, 128
C_SIZE, Y_SIZE = 3, 2
N_CORES = 8
T_LOC = T * B // N_CORES  # 256 targets per core
T_BLK = 128
N_BLK = T_LOC // T_BLK  # 2
N_CC = C_CTX // 128  # 4 context chunks
ACT_EVERY = 4  # every ACT_EVERY-th target handled by ScalarE, rest VectorE

F32 = mybir.dt.float32
BF16 = mybir.dt.bfloat16
AF = mybir.ActivationFunctionType
ALU = mybir.AluOpType

_CACHE: dict = {}


def _build(reps: int = 1):
    nc = bacc.Bacc("TRN2", target_bir_lowering=False)

    encT = nc.dram_tensor("encT", [6, C_CTX], F32, kind="ExternalInput")
    tgtT = nc.dram_tensor("tgtT", [4, T_LOC], F32, kind="ExternalInput")
    w_in = nc.dram_tensor("w_in", [6, H], F32, kind="ExternalInput")
    w_ctx = nc.dram_tensor("w_ctx", [4, H], F32, kind="ExternalInput")
    w_tgt = nc.dram_tensor("w_tgt", [H, H], BF16, kind="ExternalInput")
    ones_c = nc.dram_tensor("ones_c", [H, 1], BF16, kind="ExternalInput")
    btgt = nc.dram_tensor("btgt", [H, H], F32, kind="ExternalInput")
    out_d = nc.dram_tensor("out", [T_LOC, H], F32, kind="ExternalOutput")

    with tile.TileContext(nc) as tc:
        with (
            tc.tile_pool(name="consts", bufs=1) as consts,
            tc.tile_pool(name="absd", bufs=6) as absd_pool,
            tc.tile_pool(name="work", bufs=2) as work,
            tc.tile_pool(name="stats", bufs=2) as stats,
            tc.tile_pool(name="pbig", bufs=3, space="PSUM") as pbig,
            tc.tile_pool(name="psmall", bufs=3, space="PSUM") as psmall,
        ):
            # ---- load constants ----
            encT_sb = consts.tile([6, C_CTX], F32)
            tgtT_sb = consts.tile([4, T_LOC], F32)
            w_in_sb = consts.tile([6, H], F32)
            w_ctx_sb = consts.tile([4, H], F32)
            w_tgt_sb = consts.tile([H, H], BF16)
            ones_sb = consts.tile([H, 1], BF16)
            btgt_sb = consts.tile([H, H], F32)
            for sb, dr in (
                (encT_sb, encT), (tgtT_sb, tgtT), (w_in_sb, w_in),
                (w_ctx_sb, w_ctx), (w_tgt_sb, w_tgt), (ones_sb, ones_c),
                (btgt_sb, btgt),
            ):
                nc.sync.dma_start(out=sb[:], in_=dr[:])

            # ---- projections ----
            # keyT[h, c] = sum_i W_ctx[i,h]*ctx_c[c,i] + b_ctx[h]
            psum_key = pbig.tile([H, C_CTX], F32, tag="big")
            nc.tensor.matmul(psum_key[:], w_ctx_sb[:], encT_sb[0:4, :],
                             start=True, stop=True)
            key_half = consts.tile([H, C_CTX], BF16)
            nc.vector.tensor_scalar(out=key_half[:], in0=psum_key[:],
                                    scalar1=0.5, scalar2=None, op0=ALU.mult)

            # qT[h, t]; nq = -0.5 * qT  (per-partition bias for the abs pass)
            psum_q = psmall.tile([H, T_LOC], F32, tag="small")
            nc.tensor.matmul(psum_q[:], w_ctx_sb[:], tgtT_sb[:],
                             start=True, stop=True)
            nq = consts.tile([H, T_LOC], F32)
            nc.vector.tensor_scalar(out=nq[:], in0=psum_q[:],
                                    scalar1=-0.5, scalar2=None, op0=ALU.mult)

            # value[c, h] in 4 chunks of 128 context rows
            psum_val = pbig.tile([128, N_CC, H], F32, tag="big")
            for cc in range(N_CC):
                nc.tensor.matmul(psum_val[:, cc, :],
                                 encT_sb[:, cc * 128:(cc + 1) * 128],
                                 w_in_sb[:], start=True, stop=True)
            value_bf = consts.tile([128, N_CC, H], BF16)
            nc.scalar.copy(out=value_bf[:], in_=psum_val[:])

            # ---- main loop: per 128-target block ----
            # (reps > 1 repeats identical work for device-time measurement)
            for blk in [b for _ in range(reps) for b in range(N_BLK)]:
                # S^T[c, (cc, t)] accumulated one column per target
                psum_sT = pbig.tile([128, N_CC, T_BLK], F32, tag="big")
                for ti in range(T_BLK):
                    t = blk * T_BLK + ti
                    a = absd_pool.tile([H, C_CTX], BF16, tag="absd")
                    if t % ACT_EVERY == 0:
                        # ScalarE path: |key_half + (-q_half)| in one op
                        nc.scalar.activation(out=a[:], in_=key_half[:],
                                             func=AF.Abs,
                                             bias=nq[:, t:t + 1], scale=1.0)
                    else:
                        # VectorE path: d = key_half + nq[t] (4x mode), then
                        # clear the bf16 sign bit via a uint16 view for |d|
                        d = absd_pool.tile([H, C_CTX], BF16, tag="d")
                        nc.vector.tensor_scalar(out=d[:], in0=key_half[:],
                                                scalar1=nq[:, t:t + 1],
                                                scalar2=None, op0=ALU.add)
                        nc.vector.tensor_scalar(
                            out=a[:].bitcast(mybir.dt.uint16),
                            in0=d[:].bitcast(mybir.dt.uint16),
                            scalar1=0x7FFF, scalar2=None,
                            op0=ALU.bitwise_and)
                    for cc in range(N_CC):
                        nc.tensor.matmul(psum_sT[:, cc, ti:ti + 1],
                                         a[:, cc * 128:(cc + 1) * 128],
                                         ones_sb[:], start=True, stop=True)

                # e^T = exp(-S^T)   (scores <= 0 -> no overflow, no max pass)
                eT_sb = work.tile([128, N_CC, T_BLK], BF16, tag="eT")
                nc.scalar.activation(out=eT_sb[:], in_=psum_sT[:],
                                     func=AF.Exp, scale=-1.0)

                # Z[t] = sum_c e[c, t]  (eT stationary, ones moving)
                psum_z = psmall.tile([T_BLK, 1], F32, tag="small")
                for cc in range(N_CC):
                    nc.tensor.matmul(psum_z[:], eT_sb[:, cc, :], ones_sb[:],
                                     start=(cc == 0), stop=(cc == N_CC - 1))
                invz = stats.tile([T_BLK, 1], F32, tag="invz")
                nc.vector.reciprocal(invz[:], psum_z[:])

                # rep^T[h, t] = sum_c value[c, h] * e[c, t]
                psum_rep = psmall.tile([H, T_BLK], F32, tag="small")
                for cc in range(N_CC):
                    nc.tensor.matmul(psum_rep[:], value_bf[:, cc, :],
                                     eT_sb[:, cc, :],
                                     start=(cc == 0), stop=(cc == N_CC - 1))
                repT_sb = work.tile([H, T_BLK], BF16, tag="repT")
                nc.scalar.copy(out=repT_sb[:], in_=psum_rep[:])

                # out[t, h2] = (rep^T.T @ W_tgt)[t, h2] * invZ[t] + b_tgt[h2]
                psum_o = psmall.tile([T_BLK, H], F32, tag="small")
                nc.tensor.matmul(psum_o[:], repT_sb[:], w_tgt_sb[:],
                                 start=True, stop=True)
                out_sb = work.tile([T_BLK, H], F32, tag="osb")
                nc.vector.scalar_tensor_tensor(out=out_sb[:], in0=psum_o[:],
                                               scalar=invz[:, 0:1],
                                               in1=btgt_sb[:],
                                               op0=ALU.mult, op1=ALU.add)
                nc.sync.dma_start(
                    out=out_d[blk * T_BLK:(blk + 1) * T_BLK, :],
                    in_=out_sb[:])

    nc.compile()
    return nc


def _get_nc(reps: int = 1):
    key = f"nc{reps}"
    if key not in _CACHE:
        _CACHE[key] = _build(reps)
    return _CACHE[key]


def _in_maps(context_x, context_y, target_x, W_in, b_in, W_ctx, b_ctx,
             W_tgt, b_tgt):
    f32 = np.float32
    bf16 = ml_dtypes.bfloat16
    w_in_ext = np.concatenate(
        [W_in[:C_SIZE], b_in[None, :], W_in[C_SIZE:]], axis=0).astype(f32)
    w_ctx_ext = np.concatenate([W_ctx, b_ctx[None, :]], axis=0).astype(f32)
    w_tgt_bf = np.ascontiguousarray(W_tgt).astype(bf16)
    ones_col = np.ones((H, 1), dtype=bf16)
    btgt_bc = np.ascontiguousarray(
        np.broadcast_to(b_tgt[None, :], (H, H))).astype(f32)

    ones_row_c = np.ones((1, C_CTX), f32)
    ones_row_t = np.ones((1, T_LOC), f32)
    maps = []
    for core in range(N_CORES):
        b = core // 2
        th = core % 2
        ctx_cT = np.ascontiguousarray(context_x[b, :, :C_SIZE].T).astype(f32)
        ctx_yT = np.ascontiguousarray(context_y[b].T).astype(f32)
        encT = np.concatenate([ctx_cT, ones_row_c, ctx_yT], axis=0)
        tgt = target_x[b, th * T_LOC:(th + 1) * T_LOC, :C_SIZE]
        tgtT = np.concatenate(
            [np.ascontiguousarray(tgt.T).astype(f32), ones_row_t], axis=0)
        maps.append({
            "encT": np.ascontiguousarray(encT),
            "tgtT": np.ascontiguousarray(tgtT),
            "w_in": w_in_ext,
            "w_ctx": w_ctx_ext,
            "w_tgt": w_tgt_bf,
            "ones_c": ones_col,
            "btgt": btgt_bc,
        })
    return maps


def kernel(**inputs):
    nc = _get_nc(_CACHE.get("reps", 1))
    maps = _in_maps(**{k: np.asarray(v) for k, v in inputs.items()})
    res = run_bass_kernel_spmd(nc, maps, core_ids=list(range(N_CORES)),
                               **_CACHE.get("run_kwargs", {}))
    _CACHE["last_result"] = res
    out = np.empty((B, T, H), np.float32)
    for core in range(N_CORES):
        b = core // 2
        th = core % 2
        out[b, th * T_LOC:(th + 1) * T_LOC, :] = res.results[core]["out"]
    return out


# revision 11
# speedup vs baseline: 36.3578x; 36.3578x over previous
"""Trainium2 Bass kernel for nn_LocationEncoder (L1-distance attention).

Math (per batch element b):
    key   = ctx_c @ W_ctx + b_ctx                  [C, H]
    query = tgt_c @ W_ctx + b_ctx                  [T, H]
    value = [ctx_c, ctx_y] @ W_in + b_in           [C, H]
    S[t, c]  = sum_h |0.5*(key[c,h] - query[t,h])|  (= -score, always >= 0)
    w = softmax(-S, axis=c);  out = (w @ value) @ W_tgt + b_tgt

Sharding: 8 cores = (4 batches) x (2 halves of T).  Each core handles
T_LOC = 256 targets against the full C = 512 context of its batch.

This execution environment has a large per-instruction dispatch overhead
(~33 us/instruction measured), so the kernel is built from ~37 very large
instructions:
  - key/value via 4 fused matmuls ([key_half | value] share the lhsT),
    nq = -0.5*query via 2 matmuls; nq is bounced through DRAM and
    broadcast-DMA'd to all 128 partitions (q_rep).
  - d[c, t, h] = key_half[c, h] + nq[t, h]: ONE tensor_tensor add per
    128-c chunk with 3D broadcast APs, FD = 256*128 = 32768 elements.
  - S^T[c, t] = sum_h |d|: ONE segmented tensor_reduce (axis=X,
    apply_absolute_value) per chunk.
  - softmax without max-subtraction (scores <= 0 cannot overflow exp):
    one Exp over all chunks, Z row via 4 ones-matmuls, reciprocal,
    broadcast-DMA of 1/Z, one multiply -> normalized weights.
  - rep^T = value.T-style accumulation (4 matmuls), out = rep^T.T @ W_tgt
    (2 matmuls) + b_tgt (one add), one output DMA.
"""

import numpy as np
import ml_dtypes

import concourse.bass as bass
import concourse.mybir as mybir
import concourse.tile as tile
from concourse import bacc
from concourse.bass_utils import run_bass_kernel_spmd

B, C_CTX, T, H = 4, 512, 512, 128
C_SIZE, Y_SIZE = 3, 2
N_CORES = 8
T_LOC = T * B // N_CORES  # 256 targets per core
N_CC = C_CTX // 128  # 4 context chunks
N_TB = T_LOC // 128  # 2 target half-blocks

F32 = mybir.dt.float32
BF16 = mybir.dt.bfloat16
AF = mybir.ActivationFunctionType
ALU = mybir.AluOpType

# blob_f32 column offsets
O_LHS = 0        # [6, 512]  rows: ctx_cT(3), ones(1), ctx_yT(2)
O_TGT = 512      # [4, 256]  rows: tgt_cT(3), ones(1)
O_KV = 768       # [6, 256]  [:, :128]=key rhs (0.5W_ctx,0.5b,0,0) [:,128:]=W_in_ext
O_NQ = 1024      # [4, 128]  -0.5*W_ctx, -0.5*b_ctx
O_BT = 1152      # [128, 128] b_tgt broadcast
F32_COLS = 1280

_CACHE: dict = {}


def _build(reps: int = 1):
    nc = bacc.Bacc("TRN2", target_bir_lowering=False)

    blob_f = nc.dram_tensor("blob_f", [128, F32_COLS], F32,
                            kind="ExternalInput")
    blob_b = nc.dram_tensor("blob_b", [128, H + 1], BF16,
                            kind="ExternalInput")
    nq_dram = nc.dram_tensor("nq_dram", [T_LOC, H], BF16)
    zs_dram = nc.dram_tensor("zs_dram", [1, T_LOC], F32)
    out_d = nc.dram_tensor("out", [T_LOC, H], F32, kind="ExternalOutput")

    with tile.TileContext(nc) as tc:
        with (
            tc.tile_pool(name="consts", bufs=1) as consts,
            tc.tile_pool(name="work", bufs=1) as work,
            tc.tile_pool(name="ps", bufs=1, space="PSUM") as ps,
        ):
            # ---- load constants ----
            sb_f = consts.tile([128, F32_COLS], F32)
            sb_b = consts.tile([128, H + 1], BF16)
            nc.sync.dma_start(out=sb_f[:], in_=blob_f[:])
            nc.sync.dma_start(out=sb_b[:], in_=blob_b[:])
            w_tgt = sb_b[:, 0:H]
            ones_col = sb_b[:, H:H + 1]
            btgt = sb_f[:, O_BT:O_BT + 128]

            # ---- projections ----
            # kv[c, (key_half | value)] per 128-c chunk
            psum_kv = ps.tile([128, N_CC, 256], F32, tag="kv")
            for cc in range(N_CC):
                nc.tensor.matmul(
                    psum_kv[:, cc, :],
                    sb_f[0:6, O_LHS + cc * 128:O_LHS + (cc + 1) * 128],
                    sb_f[0:6, O_KV:O_KV + 256],
                    start=True, stop=True)
            kv_bf = consts.tile([128, N_CC, 256], BF16)
            nc.scalar.copy(out=kv_bf[:], in_=psum_kv[:])

            # nq[t, h] = -0.5 * query
            psum_nq = ps.tile([128, N_TB, H], F32, tag="nq")
            for j in range(N_TB):
                nc.tensor.matmul(
                    psum_nq[:, j, :],
                    sb_f[0:4, O_TGT + j * 128:O_TGT + (j + 1) * 128],
                    sb_f[0:4, O_NQ:O_NQ + 128],
                    start=True, stop=True)
            nq_bf = consts.tile([128, N_TB, H], BF16)
            nc.scalar.copy(out=nq_bf[:], in_=psum_nq[:])

            # bounce nq through DRAM, then broadcast to all 128 partitions
            nq_rows = nq_dram[:].rearrange("(j t) h -> t j h", j=N_TB)
            nc.sync.dma_start(out=nq_rows, in_=nq_bf[:])
            q_rep = consts.tile([128, T_LOC * H], BF16)
            q_flat = nq_dram[:].flatten().partition_broadcast(128)
            nc.sync.dma_start(out=q_rep[:], in_=q_flat)

            for _ in range(reps):
                # ---- scores: d = key_half + nq ; S = sum_h |d| ----
                s_all = work.tile([128, N_CC, T_LOC], F32, tag="s")
                d3 = work.tile([128, T_LOC, H], BF16, tag="d3")
                for cc in range(N_CC):
                    key_b = kv_bf[:, cc, 0:H].unsqueeze(1).broadcast_to(
                        [128, T_LOC, H])
                    nc.vector.tensor_tensor(
                        out=d3[:], in0=key_b,
                        in1=q_rep[:].rearrange("p (t h) -> p t h", h=H),
                        op=ALU.add)
                    nc.vector.tensor_reduce(
                        out=s_all[:, cc, :], in_=d3[:],
                        axis=mybir.AxisListType.X, op=ALU.add,
                        apply_absolute_value=True)

                # ---- softmax (no max-subtraction needed) ----
                eT = work.tile([128, N_CC, T_LOC], BF16, tag="eT")
                nc.scalar.activation(out=eT[:], in_=s_all[:], func=AF.Exp,
                                     scale=-1.0)
                psum_z = ps.tile([1, T_LOC], F32, tag="z")
                for cc in range(N_CC):
                    nc.tensor.matmul(psum_z[:], ones_col, eT[:, cc, :],
                                     start=(cc == 0), stop=(cc == N_CC - 1))
                invz = work.tile([1, T_LOC], F32, tag="invz")
                nc.vector.reciprocal(invz[:], psum_z[:])
                invz_rep = work.tile([128, T_LOC], F32, tag="invzr")
                nc.sync.dma_start(out=zs_dram[:], in_=invz[:])
                nc.sync.dma_start(
                    out=invz_rep[:],
                    in_=zs_dram[:].flatten().partition_broadcast(128))
                e_norm = work.tile([128, N_CC, T_LOC], BF16, tag="en")
                nc.vector.tensor_tensor(
                    out=e_norm[:], in0=eT[:],
                    in1=invz_rep[:].unsqueeze(1).broadcast_to(
                        [128, N_CC, T_LOC]),
                    op=ALU.mult)

                # ---- rep^T[h, t] and output ----
                psum_rep = ps.tile([H, T_LOC], F32, tag="rep")
                for cc in range(N_CC):
                    nc.tensor.matmul(psum_rep[:],
                                     kv_bf[:, cc, 128:256],
                                     e_norm[:, cc, :],
                                     start=(cc == 0), stop=(cc == N_CC - 1))
                rep_bf = work.tile([H, T_LOC], BF16, tag="repbf")
                nc.scalar.copy(out=rep_bf[:], in_=psum_rep[:])

                psum_o = ps.tile([128, N_TB, H], F32, tag="o")
                for j in range(N_TB):
                    nc.tensor.matmul(psum_o[:, j, :],
                                     rep_bf[:, j * 128:(j + 1) * 128],
                                     w_tgt, start=True, stop=True)
                out_sb = work.tile([128, N_TB, H], F32, tag="osb")
                nc.vector.tensor_tensor(
                    out=out_sb[:], in0=psum_o[:],
                    in1=btgt.unsqueeze(1).broadcast_to([128, N_TB, 128]),
                    op=ALU.add)
                out_rows = out_d[:].rearrange("(j t) h -> t j h", j=N_TB)
                nc.sync.dma_start(out=out_rows, in_=out_sb[:])

    nc.compile()
    return nc


def _get_nc(reps: int = 1):
    key = f"nc{reps}"
    if key not in _CACHE:
        _CACHE[key] = _build(reps)
    return _CACHE[key]


def _in_maps(context_x, context_y, target_x, W_in, b_in, W_ctx, b_ctx,
             W_tgt, b_tgt):
    f32 = np.float32
    bf16 = ml_dtypes.bfloat16

    blob_b = np.zeros((128, H + 1), bf16)
    blob_b[:, 0:H] = W_tgt.astype(bf16)
    blob_b[:, H] = bf16(1.0)

    kv_rhs = np.zeros((6, 256), f32)
    kv_rhs[0:C_SIZE, 0:H] = 0.5 * W_ctx
    kv_rhs[C_SIZE, 0:H] = 0.5 * b_ctx
    kv_rhs[0:C_SIZE, H:256] = W_in[0:C_SIZE]
    kv_rhs[C_SIZE, H:256] = b_in
    kv_rhs[C_SIZE + 1:6, H:256] = W_in[C_SIZE:]

    nq_rhs = np.zeros((4, H), f32)
    nq_rhs[0:C_SIZE] = -0.5 * W_ctx
    nq_rhs[C_SIZE] = -0.5 * b_ctx

    maps = []
    for core in range(N_CORES):
        b = core // 2
        th = core % 2
        blob_f = np.zeros((128, F32_COLS), f32)
        blob_f[0:C_SIZE, O_LHS:O_LHS + C_CTX] = context_x[b, :, :C_SIZE].T
        blob_f[C_SIZE, O_LHS:O_LHS + C_CTX] = 1.0
        blob_f[4:6, O_LHS:O_LHS + C_CTX] = context_y[b].T
        tgt = target_x[b, th * T_LOC:(th + 1) * T_LOC, :C_SIZE]
        blob_f[0:C_SIZE, O_TGT:O_TGT + T_LOC] = tgt.T
        blob_f[C_SIZE, O_TGT:O_TGT + T_LOC] = 1.0
        blob_f[0:6, O_KV:O_KV + 256] = kv_rhs
        blob_f[0:4, O_NQ:O_NQ + H] = nq_rhs
        blob_f[:, O_BT:O_BT + 128] = b_tgt[None, :]
        maps.append({"blob_f": blob_f, "blob_b": blob_b})
    return maps


def kernel(**inputs):
    nc = _get_nc(_CACHE.get("reps", 1))
    maps = _in_maps(**{k: np.asarray(v) for k, v in inputs.items()})
    res = run_bass_kernel_spmd(nc, maps, core_ids=list(range(N_CORES)),
                               **_CACHE.get("run_kwargs", {}))
    _CACHE["last_result"] = res
    out = np.empty((B, T, H), np.float32)
    for core in range(N_CORES):
        b = core // 2
        th = core % 2
        out[b, th * T_LOC:(th + 1) * T_LOC, :] = res.results[core]["out"]
    return out


# revision 13
# speedup vs baseline: 76.9977x; 2.1178x over previous
"""Trainium2 Bass kernel for nn_LocationEncoder (L1-distance attention).

Math (per batch element b):
    key   = ctx_c @ W_ctx + b_ctx                  [C, H]
    query = tgt_c @ W_ctx + b_ctx                  [T, H]
    value = [ctx_c, ctx_y] @ W_in + b_in           [C, H]
    S[t, c]  = sum_h |0.5*(key[c,h] - query[t,h])|  (= -score, always >= 0)
    w = softmax(-S, axis=c);  out = (w @ value) @ W_tgt + b_tgt

Sharding: 8 cores = (4 batches) x (2 halves of T).  Each core handles
T_LOC = 256 targets against the full C = 512 context of its batch.

This execution environment has a large per-instruction dispatch overhead
(~33 us/instruction measured), so the kernel is built from ~37 very large
instructions:
  - key/value via 4 fused matmuls ([key_half | value] share the lhsT),
    nq = -0.5*query via 2 matmuls; nq is bounced through DRAM and
    broadcast-DMA'd to all 128 partitions (q_rep).
  - d[c, t, h] = key_half[c, h] + nq[t, h]: ONE tensor_tensor add per
    128-c chunk with 3D broadcast APs, FD = 256*128 = 32768 elements.
  - S^T[c, t] = sum_h |d|: ONE segmented tensor_reduce (axis=X,
    apply_absolute_value) per chunk.
  - softmax without max-subtraction (scores <= 0 cannot overflow exp):
    one Exp over all chunks, Z row via 4 ones-matmuls, reciprocal,
    broadcast-DMA of 1/Z, one multiply -> normalized weights.
  - rep^T = value.T-style accumulation (4 matmuls), out = rep^T.T @ W_tgt
    (2 matmuls) + b_tgt (one add), one output DMA.
"""

import numpy as np
import ml_dtypes

import concourse.bass as bass
import concourse.mybir as mybir
import concourse.tile as tile
from concourse import bacc
from concourse.bass_utils import run_bass_kernel_spmd

B, C_CTX, T, H = 4, 512, 512, 128
C_SIZE, Y_SIZE = 3, 2
N_CORES = 8
T_LOC = T * B // N_CORES  # 256 targets per core
N_CC = C_CTX // 128  # 4 context chunks
N_TB = T_LOC // 128  # 2 target half-blocks

F32 = mybir.dt.float32
BF16 = mybir.dt.bfloat16
AF = mybir.ActivationFunctionType
ALU = mybir.AluOpType

# blob_f32 column offsets
O_LHS = 0        # [6, 512]  rows: ctx_cT(3), ones(1), ctx_yT(2)
O_TGT = 512      # [4, 256]  rows: tgt_cT(3), ones(1)
O_KV = 768       # [6, 256]  [:, :128]=key rhs (0.5W_ctx,0.5b,0,0) [:,128:]=W_in_ext
O_NQ = 1024      # [4, 128]  -0.5*W_ctx, -0.5*b_ctx
O_BT = 1152      # [128, 128] b_tgt broadcast
F32_COLS = 1280

_CACHE: dict = {}


def _build(reps: int = 1):
    nc = bacc.Bacc("TRN2", target_bir_lowering=False)

    blob_f = nc.dram_tensor("blob_f", [128, F32_COLS], F32,
                            kind="ExternalInput")
    blob_b = nc.dram_tensor("blob_b", [128, H + 1], BF16,
                            kind="ExternalInput")
    nq_dram = nc.dram_tensor("nq_dram", [T_LOC, H], BF16)
    zs_dram = nc.dram_tensor("zs_dram", [1, T_LOC], F32)
    out_d = nc.dram_tensor("out", [T_LOC, H], F32, kind="ExternalOutput")

    with tile.TileContext(nc) as tc:
        with (
            tc.tile_pool(name="consts", bufs=1) as consts,
            tc.tile_pool(name="work", bufs=1) as work,
            tc.tile_pool(name="ps", bufs=1, space="PSUM") as ps,
        ):
            # ---- load constants ----
            sb_f = consts.tile([128, F32_COLS], F32)
            sb_b = consts.tile([128, H + 1], BF16)
            nc.sync.dma_start(out=sb_f[:], in_=blob_f[:])
            nc.sync.dma_start(out=sb_b[:], in_=blob_b[:])
            w_tgt = sb_b[:, 0:H]
            ones_col = sb_b[:, H:H + 1]
            btgt = sb_f[:, O_BT:O_BT + 128]

            # ---- projections ----
            # kv[c, (key_half | value)] per 128-c chunk
            psum_kv = ps.tile([128, N_CC, 256], F32, tag="kv")
            for cc in range(N_CC):
                nc.tensor.matmul(
                    psum_kv[:, cc, :],
                    sb_f[0:6, O_LHS + cc * 128:O_LHS + (cc + 1) * 128],
                    sb_f[0:6, O_KV:O_KV + 256],
                    start=True, stop=True)
            kv_bf = consts.tile([128, N_CC, 256], BF16)
            nc.vector.tensor_copy(out=kv_bf[:], in_=psum_kv[:])

            # nq[t, h] = -0.5 * query
            psum_nq = ps.tile([128, N_TB, H], F32, tag="nq")
            for j in range(N_TB):
                nc.tensor.matmul(
                    psum_nq[:, j, :],
                    sb_f[0:4, O_TGT + j * 128:O_TGT + (j + 1) * 128],
                    sb_f[0:4, O_NQ:O_NQ + 128],
                    start=True, stop=True)
            nq_bf = consts.tile([128, N_TB, H], BF16)
            nc.vector.tensor_copy(out=nq_bf[:], in_=psum_nq[:])

            # bounce nq through DRAM, then broadcast to all 128 partitions
            nq_rows = nq_dram[:].rearrange("(j t) h -> t j h", j=N_TB)
            nc.sync.dma_start(out=nq_rows, in_=nq_bf[:])
            q_rep = consts.tile([128, T_LOC * H], BF16)
            q_flat = nq_dram[:].flatten().partition_broadcast(128)
            nc.sync.dma_start(out=q_rep[:], in_=q_flat)

            for _ in range(reps):
                # ---- scores: d = key_half + nq ; S = sum_h |d| ----
                s_all = work.tile([128, N_CC, T_LOC], F32, tag="s")
                d3 = work.tile([128, T_LOC, H], BF16, tag="d3")
                for cc in range(N_CC):
                    key_b = kv_bf[:, cc, 0:H].unsqueeze(1).broadcast_to(
                        [128, T_LOC, H])
                    nc.vector.tensor_tensor(
                        out=d3[:], in0=key_b,
                        in1=q_rep[:].rearrange("p (t h) -> p t h", h=H),
                        op=ALU.add)
                    nc.vector.tensor_reduce(
                        out=s_all[:, cc, :], in_=d3[:],
                        axis=mybir.AxisListType.X, op=ALU.add,
                        apply_absolute_value=True)

                # ---- softmax (no max-subtraction needed) ----
                eT = work.tile([128, N_CC, T_LOC], BF16, tag="eT")
                nc.scalar.activation(out=eT[:], in_=s_all[:], func=AF.Exp,
                                     scale=-1.0)
                psum_z = ps.tile([1, T_LOC], F32, tag="z")
                z_out = psum_z[:].unsqueeze(1).broadcast_to([1, 2, T_LOC])
                for half in range(2):
                    nc.tensor.matmul(z_out, ones_col,
                                     eT[:, 2 * half:2 * half + 2, :],
                                     start=(half == 0), stop=(half == 1))
                invz = work.tile([1, T_LOC], F32, tag="invz")
                nc.vector.reciprocal(invz[:], psum_z[:])
                invz_rep = work.tile([128, T_LOC], F32, tag="invzr")
                nc.sync.dma_start(out=zs_dram[:], in_=invz[:])
                nc.sync.dma_start(
                    out=invz_rep[:],
                    in_=zs_dram[:].flatten().partition_broadcast(128))
                e_norm = work.tile([128, N_CC, T_LOC], BF16, tag="en")
                nc.vector.tensor_tensor(
                    out=e_norm[:], in0=eT[:],
                    in1=invz_rep[:].unsqueeze(1).broadcast_to(
                        [128, N_CC, T_LOC]),
                    op=ALU.mult)

                # ---- rep^T[h, t] and output ----
                psum_rep = ps.tile([H, T_LOC], F32, tag="rep")
                for cc in range(N_CC):
                    nc.tensor.matmul(psum_rep[:],
                                     kv_bf[:, cc, 128:256],
                                     e_norm[:, cc, :],
                                     start=(cc == 0), stop=(cc == N_CC - 1))
                rep_bf = work.tile([H, T_LOC], BF16, tag="repbf")
                nc.vector.tensor_copy(out=rep_bf[:], in_=psum_rep[:])

                psum_o = ps.tile([128, N_TB, H], F32, tag="o")
                for j in range(N_TB):
                    nc.tensor.matmul(psum_o[:, j, :],
                                     rep_bf[:, j * 128:(j + 1) * 128],
                                     w_tgt, start=True, stop=True)
                out_sb = work.tile([128, N_TB, H], F32, tag="osb")
                nc.vector.tensor_tensor(
                    out=out_sb[:], in0=psum_o[:],
                    in1=btgt.unsqueeze(1).broadcast_to([128, N_TB, 128]),
                    op=ALU.add)
                out_rows = out_d[:].rearrange("(j t) h -> t j h", j=N_TB)
                nc.sync.dma_start(out=out_rows, in_=out_sb[:])

    nc.compile()
    return nc


def _get_nc(reps: int = 1):
    key = f"nc{reps}"
    if key not in _CACHE:
        _CACHE[key] = _build(reps)
    return _CACHE[key]


def _in_maps(context_x, context_y, target_x, W_in, b_in, W_ctx, b_ctx,
             W_tgt, b_tgt):
    f32 = np.float32
    bf16 = ml_dtypes.bfloat16

    blob_b = np.zeros((128, H + 1), bf16)
    blob_b[:, 0:H] = W_tgt.astype(bf16)
    blob_b[:, H] = bf16(1.0)

    kv_rhs = np.zeros((6, 256), f32)
    kv_rhs[0:C_SIZE, 0:H] = 0.5 * W_ctx
    kv_rhs[C_SIZE, 0:H] = 0.5 * b_ctx
    kv_rhs[0:C_SIZE, H:256] = W_in[0:C_SIZE]
    kv_rhs[C_SIZE, H:256] = b_in
    kv_rhs[C_SIZE + 1:6, H:256] = W_in[C_SIZE:]

    nq_rhs = np.zeros((4, H), f32)
    nq_rhs[0:C_SIZE] = -0.5 * W_ctx
    nq_rhs[C_SIZE] = -0.5 * b_ctx

    maps = []
    for core in range(N_CORES):
        b = core // 2
        th = core % 2
        blob_f = np.zeros((128, F32_COLS), f32)
        blob_f[0:C_SIZE, O_LHS:O_LHS + C_CTX] = context_x[b, :, :C_SIZE].T
        blob_f[C_SIZE, O_LHS:O_LHS + C_CTX] = 1.0
        blob_f[4:6, O_LHS:O_LHS + C_CTX] = context_y[b].T
        tgt = target_x[b, th * T_LOC:(th + 1) * T_LOC, :C_SIZE]
        blob_f[0:C_SIZE, O_TGT:O_TGT + T_LOC] = tgt.T
        blob_f[C_SIZE, O_TGT:O_TGT + T_LOC] = 1.0
        blob_f[0:6, O_KV:O_KV + 256] = kv_rhs
        blob_f[0:4, O_NQ:O_NQ + H] = nq_rhs
        blob_f[:, O_BT:O_BT + 128] = b_tgt[None, :]
        maps.append({"blob_f": blob_f, "blob_b": blob_b})
    return maps


def kernel(**inputs):
    nc = _get_nc(_CACHE.get("reps", 1))
    maps = _in_maps(**{k: np.asarray(v) for k, v in inputs.items()})
    res = run_bass_kernel_spmd(nc, maps, core_ids=list(range(N_CORES)),
                               **_CACHE.get("run_kwargs", {}))
    _CACHE["last_result"] = res
    out = np.empty((B, T, H), np.float32)
    for core in range(N_CORES):
        b = core // 2
        th = core % 2
        out[b, th * T_LOC:(th + 1) * T_LOC, :] = res.results[core]["out"]
    return out
